# revision 1
# baseline (speedup 1.0000x reference)
"""Trainium2 Bass kernel for the all-pairs cosine-similarity loss.

Reference computes:  loss = mean_{i<j}(1 - cos(f_i, f_j))
Closed form used here (mathematically identical for nonzero rows):
    u_i = f_i / ||f_i||           (normalized rows)
    g   = sum_i u_i               (D-vector)
    sum_{i<j} cos(f_i,f_j) = (||g||^2 - N) / 2
    loss = 1 - (||g||^2 - N) / (2 * num_pairs)

This turns an O(N^2 D) matmul problem into an O(N D) memory-bound pass:
each core streams its 512-row shard once (cast to bf16 during the DMA),
computes row norms (ACT square+accum), does a weighted row-sum on the
tensor engine (w = 1/||f_i|| stationary, bf16), AllGathers the 8
partial [1024] vectors, and finishes the scalar on-device.

bf16 note: the matmul operands are bf16 but every accumulation is fp32
(PSUM / accum_out).  The loss is 1 + O(1e-5) and the bf16 rounding of
unit-normalized rows perturbs it by ~1e-7 — far below the fp32
rounding noise of the reference's own 16M-element reduction.
"""

import numpy as np

N = 4096
D = 1024
N_CORES = 8
ROWS = N // N_CORES          # 512 rows per core
P = 128                      # SBUF partitions
T = ROWS // P                # 4 row-tiles of [128, D] per core
NUM_PAIRS = N * (N - 1) // 2

_LOSS_SCALE = -1.0 / (2.0 * NUM_PAIRS)
_LOSS_BIAS = 1.0 + N / (2.0 * NUM_PAIRS)

_built = None


def _build(collective: bool = True):
    import concourse.bacc as bacc
    import concourse.mybir as mybir
    import concourse.tile as tile

    f32 = mybir.dt.float32
    bf16 = mybir.dt.bfloat16
    nc = bacc.Bacc(
        "TRN2", target_bir_lowering=False, debug=False, num_devices=N_CORES
    )

    feats = nc.dram_tensor("feats", [ROWS, D], f32, kind="ExternalInput")
    loss_out = nc.dram_tensor("loss", [1, 1], f32, kind="ExternalOutput")
    # Internal DRAM bounce buffers for the collective (I/O tensors are not
    # legal collective operands; output must be in the Shared scratchpad).
    g_local = nc.dram_tensor("g_local", [1, D], bf16)
    g_all = nc.dram_tensor("g_all", [N_CORES, D], bf16, addr_space="Shared")

    with tile.TileContext(nc) as tc:
        with (
            tc.tile_pool(name="pool", bufs=1) as pool,
            tc.tile_pool(name="psum", bufs=1, space="PSUM") as psum,
        ):
            # Warm both ACT function-table sets (Square / Sqrt+Copy) while
            # the input DMAs stream — otherwise the 1.3us table load for
            # Sqrt lands on the critical path between squares and matmuls.
            dummy = pool.tile([1, 1], f32, tag="dummy")
            nc.gpsimd.memset(dummy[:], 1.0)
            nc.scalar.square(dummy[:], dummy[:])
            nc.scalar.sqrt(dummy[:], dummy[:])

            # Load + cast f32 -> bf16 during the DMA (SWDGE handles the
            # dtype conversion inline).
            fview = feats.ap().rearrange("(t p) d -> t p d", p=P)
            ftiles = []
            for t in range(T):
                ft = pool.tile([P, D], bf16, tag=f"f{t}", name=f"ft{t}")
                nc.gpsimd.dma_start(ft[:], fview[t])
                ftiles.append(ft)

            # Per-tile chains: square+rowsum (ACT, fp32 accum) -> sqrt (ACT)
            # -> reciprocal (DVE) -> bf16 cast (DVE) -> PE matmul pair.
            # Per-tile (not batched) so tile t's matmuls start as soon as
            # its own norm is ready instead of waiting on all 4 squares.
            # NB: vector.tensor_tensor_reduce crashes the NRT worker on
            # this runtime — keep to ACT/standard DVE instructions.
            sq = pool.tile([P, T], f32, tag="sq")
            nrm = pool.tile([P, T], f32, tag="nrm")
            w = pool.tile([P, T], f32, tag="w")
            wb = pool.tile([P, T], bf16, tag="wb")
            sc_a = pool.tile([P, D], bf16, tag="sc_a")
            gp = psum.tile([1, D], f32, tag="gp")
            for t in range(T):
                ts = slice(t, t + 1)
                nc.scalar.activation(
                    sc_a[:],
                    ftiles[t][:],
                    mybir.ActivationFunctionType.Square,
                    accum_out=sq[:, ts],
                )
                nc.scalar.sqrt(nrm[:, ts], sq[:, ts])
                nc.vector.reciprocal(w[:, ts], nrm[:, ts])
                nc.vector.tensor_copy(wb[:, ts], w[:, ts])
                for h in range(2):
                    nc.tensor.matmul(
                        gp[:, h * 512 : (h + 1) * 512],
                        wb[:, ts],
                        ftiles[t][:, h * 512 : (h + 1) * 512],
                        start=(t == 0),
                        stop=(t == T - 1),
                    )

            # PSUM -> SBUF (split across ACT+DVE, casting to bf16 so the
            # collective ships 2KB/rank) -> DRAM (dma_start can't source
            # PSUM). g ~ O(10) per component, so bf16 here costs ~1e-7 on
            # the final loss.
            gs = pool.tile([1, D], bf16, tag="gs")
            nc.scalar.copy(gs[:, 0:512], gp[:, 0:512])
            nc.vector.tensor_copy(gs[:, 512:D], gp[:, 512:D])
            nc.sync.dma_start(g_local.ap(), gs[:])

            if collective:
                nc.gpsimd.collective_compute(
                    "AllGather",
                    mybir.AluOpType.bypass,
                    replica_groups=[list(range(N_CORES))],
                    ins=[g_local.ap().opt()],
                    outs=[g_all.ap().opt()],
                )
            else:
                # timing-model variant (TimelineSim can't simulate
                # collectives): stand-in DMA with the same data deps
                nc.sync.dma_start(g_all.ap()[0:1], g_local.ap())

            # Bring the 8 partials in as [8, D] bf16 (cast on load),
            # reduce ranks on PE with a ones vector, square-reduce on ACT.
            ga = pool.tile([N_CORES, D], bf16, tag="ga")
            nc.gpsimd.dma_start(ga[:], g_all.ap())
            ones8 = pool.tile([N_CORES, 1], bf16, tag="ones8")
            nc.gpsimd.memset(ones8[:], 1.0)

            gt = psum.tile([1, D], f32, tag="gt")
            for h in range(2):
                nc.tensor.matmul(
                    gt[:, h * 512 : (h + 1) * 512],
                    ones8[:],
                    ga[:, h * 512 : (h + 1) * 512],
                    start=True,
                    stop=True,
                )

            sc_g = pool.tile([1, D], f32, tag="sc_g")
            gg = pool.tile([1, 1], f32, tag="gg")
            nc.scalar.activation(
                sc_g[:],
                gt[:],
                mybir.ActivationFunctionType.Square,
                accum_out=gg[:],
            )

            # loss = 1 - (gg - N) / (2*num_pairs)  ==  gg*scale + bias
            loss_sb = pool.tile([1, 1], f32, tag="loss_sb")
            nc.scalar.activation(
                loss_sb[:],
                gg[:],
                mybir.ActivationFunctionType.Copy,
                bias=_LOSS_BIAS,
                scale=_LOSS_SCALE,
            )
            nc.sync.dma_start(loss_out.ap(), loss_sb[:])

    nc.compile()
    return nc


def _get_nc():
    global _built
    if _built is None:
        _built = _build()
    return _built


def kernel(feats: np.ndarray) -> np.ndarray:
    from concourse import bass_utils

    nc = _get_nc()
    feats = np.ascontiguousarray(np.asarray(feats, dtype=np.float32))
    assert feats.shape == (N, D), feats.shape

    in_maps = [
        {"feats": feats[c * ROWS : (c + 1) * ROWS]} for c in range(N_CORES)
    ]
    res = bass_utils.run_bass_kernel_spmd(
        nc, in_maps, core_ids=list(range(N_CORES))
    )
    out = res.results[0]["loss"]
    return np.float32(out.reshape(())[()])



# revision 4
# speedup vs baseline: 1.1412x; 1.1412x over previous
"""Trainium2 Bass kernel for the all-pairs cosine-similarity loss.

Reference computes:  loss = mean_{i<j}(1 - cos(f_i, f_j))
Closed form used here (mathematically identical for nonzero rows):
    u_i = f_i / ||f_i||           (normalized rows)
    g   = sum_i u_i               (D-vector)
    sum_{i<j} cos(f_i,f_j) = (||g||^2 - N) / 2
    loss = 1 - (||g||^2 - N) / (2 * num_pairs)

This turns an O(N^2 D) matmul problem into an O(N D) memory-bound pass.
Per-core (512 rows):
  * 2 input DMAs stream the shard f32->bf16 (SWDGE inline cast).
  * Row norms: ACT square+accum for tiles 0-2, DVE mult+reduce for
    tile 3 (the two engines finish together; one batched sqrt/recip).
  * Weighted row-sum g = sum_i f_i/||f_i|| on PE with the F-chunks as
    STATIONARY [128x128] weights and w as the 1-column moving operand:
    32 one-column matmuls land g directly as [128 partitions x 8] in
    PSUM, so every post-matmul tensor is 128-partition shaped (one-op
    casts, small DMA descriptor counts).
  * g goes to DRAM, AllReduce(add) combines the 8 partial g's, the sum
    comes back as [8 partitions x 128] (a layout permutation, harmless:
    AllReduce is elementwise and the finisher sums all components), and
    a short ACT->PE->ACT chain writes the scalar loss (broadcast over
    128 partitions; python reads [0, 0]).

bf16 note: matmul operands are bf16 but all accumulation is fp32
(PSUM / activation accumulator).  The loss is 1 + O(1e-5); its error
is dominated by the reference's own fp32 rounding, and the bf16
rounding here perturbs it by ~1e-7.
"""

import numpy as np

N = 4096
D = 1024
N_CORES = 8
ROWS = N // N_CORES          # 512 rows per core
P = 128                      # SBUF partitions
T = ROWS // P                # 4 row-tiles of [128, D] per core
NUM_PAIRS = N * (N - 1) // 2

_LOSS_SCALE = -1.0 / (2.0 * NUM_PAIRS)
_LOSS_BIAS = 1.0 + N / (2.0 * NUM_PAIRS)

_built = None


def _build(collective: bool = True):
    import concourse.bacc as bacc
    import concourse.mybir as mybir
    import concourse.tile as tile

    f32 = mybir.dt.float32
    bf16 = mybir.dt.bfloat16
    nc = bacc.Bacc(
        "TRN2", target_bir_lowering=False, debug=False, num_devices=N_CORES
    )

    feats = nc.dram_tensor("feats", [ROWS, D], f32, kind="ExternalInput")
    # Scalar loss broadcast over 128 partitions; python reads [0, 0].
    loss_out = nc.dram_tensor("loss", [P, 1], f32, kind="ExternalOutput")
    # DRAM bounce buffers for the collective (I/O tensors are not legal
    # collective operands; output must be in the Shared scratchpad).
    g_local = nc.dram_tensor("g_local", [P, 8], bf16)
    g_all = nc.dram_tensor("g_all", [P, 8], bf16, addr_space="Shared")

    with tile.TileContext(nc) as tc:
        with (
            tc.tile_pool(name="pool", bufs=1) as pool,
            tc.tile_pool(name="psum", bufs=1, space="PSUM") as psum,
        ):
            # ---- input DMAs first: nothing else on Pool before desc-gen.
            # rows = h*256 + u*128 + p  ->  DMA h, tile t = 2h+u.
            fview = feats.ap().rearrange("(h u p) d -> h p u d", h=2, u=2, p=P)
            fab = []
            for h in range(2):
                ft = pool.tile([P, 2 * D], bf16, tag=f"f{h}", name=f"f{h}")
                nc.gpsimd.dma_start(
                    ft[:].rearrange("p (u d) -> p u d", u=2), fview[h]
                )
                fab.append(ft)

            def ftile(t, lo=0, hi=D):
                return fab[t // 2][:, (t % 2) * D + lo : (t % 2) * D + hi]

            # ---- ACT function-table warm (Square / Sqrt+Copy) while the
            # input DMAs stream.
            dummy = pool.tile([1, 1], f32, tag="dummy")
            nc.gpsimd.memset(dummy[:], 1.0)
            nc.scalar.square(dummy[:], dummy[:])
            nc.scalar.sqrt(dummy[:], dummy[:])

            ones8 = pool.tile([8, P], f32, tag="ones8")
            nc.gpsimd.memset(ones8[:], 1.0)

            # ---- row norms.  ACT: tiles 0-2 (square + free-dim accum);
            # DVE: tile 3 (mult + reduce) so both engines finish together.
            sq = pool.tile([P, T], f32, tag="sq")
            sc = pool.tile([P, D], bf16, tag="sc")
            sq3 = pool.tile([P, D], bf16, tag="sq3")
            for t in range(3):
                nc.scalar.activation(
                    sc[:],
                    ftile(t),
                    mybir.ActivationFunctionType.Square,
                    accum_out=sq[:, t : t + 1],
                )
            nc.vector.tensor_tensor(
                sq3[:], ftile(3), ftile(3), mybir.AluOpType.mult
            )
            nc.vector.tensor_reduce(
                sq[:, 3:4], sq3[:], mybir.AxisListType.X, mybir.AluOpType.add
            )

            nrm = pool.tile([P, T], f32, tag="nrm")
            w = pool.tile([P, T], f32, tag="w")
            wb = pool.tile([P, T], bf16, tag="wb")
            nc.scalar.sqrt(nrm[:], sq[:])
            nc.vector.reciprocal(w[:], nrm[:])
            nc.vector.tensor_copy(wb[:], w[:])

            # ---- g = sum_i w_i f_i as [128, 8]: F-chunks stationary,
            # w moving (1 column), PSUM accumulates over the 4 row-tiles.
            gp = psum.tile([P, 8], f32, tag="gp")
            for c in range(8):
                for t in range(T):
                    nc.tensor.matmul(
                        gp[:, c : c + 1],
                        ftile(t, c * P, (c + 1) * P),
                        wb[:, t : t + 1],
                        start=(t == 0),
                        stop=(t == T - 1),
                    )
            gs = pool.tile([P, 8], bf16, tag="gs")
            nc.vector.tensor_copy(gs[:], gp[:])

            # ---- combine the 8 per-core partials.
            nc.sync.dma_start(g_local.ap(), gs[:])
            if collective:
                nc.gpsimd.collective_compute(
                    "AllReduce",
                    mybir.AluOpType.add,
                    replica_groups=[list(range(N_CORES))],
                    ins=[g_local.ap().opt()],
                    outs=[g_all.ap().opt()],
                )
            # collective=False (timing model): the load below reads g_local
            # instead, keeping the same dependency chain; the AllReduce
            # itself is modeled externally by the test harness (its cost
            # figure is end-to-end, including the collective's own DMAs).
            gsrc = g_all if collective else g_local
            # load the summed g as [8 partitions x 128]
            ga8 = pool.tile([8, P], bf16, tag="ga8")
            nc.sync.dma_start(
                ga8[:], gsrc.ap().rearrange("(a b) n -> a (b n)", a=8)
            )

            # ---- finisher: ||g||^2 -> loss.
            scg = pool.tile([8, P], bf16, tag="scg")
            sqg = pool.tile([8, 1], f32, tag="sqg")
            nc.scalar.activation(
                scg[:],
                ga8[:],
                mybir.ActivationFunctionType.Square,
                accum_out=sqg[:],
            )
            lp = psum.tile([P, 1], f32, tag="lp")
            nc.tensor.matmul(lp[:], ones8[:], sqg[:], start=True, stop=True)
            lossb = pool.tile([P, 1], f32, tag="lossb")
            # loss = ||g||^2 * scale + bias, broadcast on 128 partitions
            nc.scalar.activation(
                lossb[:],
                lp[:],
                mybir.ActivationFunctionType.Copy,
                bias=_LOSS_BIAS,
                scale=_LOSS_SCALE,
            )
            nc.sync.dma_start(loss_out.ap(), lossb[:])

    nc.compile()
    return nc


def _get_nc():
    global _built
    if _built is None:
        _built = _build()
    return _built


def kernel(feats: np.ndarray) -> np.ndarray:
    from concourse import bass_utils

    nc = _get_nc()
    feats = np.ascontiguousarray(np.asarray(feats, dtype=np.float32))
    assert feats.shape == (N, D), feats.shape

    in_maps = [
        {"feats": feats[c * ROWS : (c + 1) * ROWS]} for c in range(N_CORES)
    ]
    res = bass_utils.run_bass_kernel_spmd(
        nc, in_maps, core_ids=list(range(N_CORES))
    )
    out = res.results[0]["loss"]
    return np.float32(out.reshape(-1)[0])


# revision 5
# speedup vs baseline: 1.1992x; 1.0509x over previous
"""Trainium2 Bass kernel for the all-pairs cosine-similarity loss.

Reference computes:  loss = mean_{i<j}(1 - cos(f_i, f_j))
Closed form used here (mathematically identical for nonzero rows):
    u_i = f_i / ||f_i||           (normalized rows)
    g   = sum_i u_i               (D-vector)
    sum_{i<j} cos(f_i,f_j) = (||g||^2 - N) / 2
    loss = 1 - (||g||^2 - N) / (2 * num_pairs)

This turns an O(N^2 D) matmul problem into an O(N D) memory-bound pass.
Per-core (512 rows):
  * 2 input DMAs stream the shard f32->bf16 (SWDGE inline cast).
  * Row norms: ACT square+accum for tiles 0-2, DVE mult+reduce for
    tile 3 (the two engines finish together; one batched sqrt/recip).
  * Weighted row-sum g = sum_i f_i/||f_i|| on PE with the F-chunks as
    STATIONARY [128x128] weights and w as the 1-column moving operand:
    32 one-column matmuls land g directly as [128 partitions x 8] in
    PSUM, so every post-matmul tensor is 128-partition shaped (one-op
    casts, small DMA descriptor counts).
  * g goes to DRAM, AllReduce(add) combines the 8 partial g's, the sum
    comes back as [8 partitions x 128] (a layout permutation, harmless:
    AllReduce is elementwise and the finisher sums all components), and
    a short ACT->PE->ACT chain writes the scalar loss (broadcast over
    128 partitions; python reads [0, 0]).

bf16 note: matmul operands are bf16 but all accumulation is fp32
(PSUM / activation accumulator).  The loss is 1 + O(1e-5); its error
is dominated by the reference's own fp32 rounding, and the bf16
rounding here perturbs it by ~1e-7.
"""

import numpy as np

N = 4096
D = 1024
N_CORES = 8
ROWS = N // N_CORES          # 512 rows per core
P = 128                      # SBUF partitions
T = ROWS // P                # 4 row-tiles of [128, D] per core
NUM_PAIRS = N * (N - 1) // 2

_LOSS_SCALE = -1.0 / (2.0 * NUM_PAIRS)
_LOSS_BIAS = 1.0 + N / (2.0 * NUM_PAIRS)

_built = None


def _build(collective: bool = True):
    import concourse.bacc as bacc
    import concourse.mybir as mybir
    import concourse.tile as tile

    f32 = mybir.dt.float32
    bf16 = mybir.dt.bfloat16
    nc = bacc.Bacc(
        "TRN2", target_bir_lowering=False, debug=False, num_devices=N_CORES
    )

    feats = nc.dram_tensor("feats", [ROWS, D], f32, kind="ExternalInput")
    # Scalar loss broadcast over 128 partitions; python reads [0, 0].
    loss_out = nc.dram_tensor("loss", [P, 1], f32, kind="ExternalOutput")
    # DRAM bounce buffers for the collective (I/O tensors are not legal
    # collective operands; output must be in the Shared scratchpad).
    g_local = nc.dram_tensor("g_local", [P, 8], bf16)
    g_all = nc.dram_tensor("g_all", [P, 8], bf16, addr_space="Shared")

    with tile.TileContext(nc) as tc:
        with (
            tc.tile_pool(name="pool", bufs=1) as pool,
            tc.tile_pool(name="psum", bufs=1, space="PSUM") as psum,
        ):
            # ---- ACT function-table warm (Square / Sqrt+Copy).  The 95ns
            # memset goes FIRST on Pool so both 1.3us table loads run while
            # the input DMAs stream instead of landing between the squares.
            dummy = pool.tile([1, 1], f32, tag="dummy")
            dummy2 = pool.tile([1, 1], f32, tag="dummy2")
            nc.gpsimd.memset(dummy[:], 1.0)
            nc.scalar.square(dummy[:], dummy[:])
            nc.scalar.sqrt(dummy2[:], dummy[:])

            # ---- input DMAs: rows = h*256 + u*128 + p -> DMA h, tile 2h+u.
            fview = feats.ap().rearrange("(h u p) d -> h p u d", h=2, u=2, p=P)
            fab = []
            for h in range(2):
                ft = pool.tile([P, 2 * D], bf16, tag=f"f{h}", name=f"f{h}")
                nc.gpsimd.dma_start(
                    ft[:].rearrange("p (u d) -> p u d", u=2), fview[h]
                )
                fab.append(ft)

            def ftile(t, lo=0, hi=D):
                return fab[t // 2][:, (t % 2) * D + lo : (t % 2) * D + hi]

            ones8 = pool.tile([8, P], f32, tag="ones8")
            nc.gpsimd.memset(ones8[:], 1.0)

            # ---- row norms.  ACT: tiles 0-2 (square + free-dim accum);
            # DVE: tile 3 (mult + reduce) so both engines finish together.
            sq = pool.tile([P, T], f32, tag="sq")
            sc = pool.tile([P, D], bf16, tag="sc")
            sq3 = pool.tile([P, D], bf16, tag="sq3")
            for t in range(3):
                nc.scalar.activation(
                    sc[:],
                    ftile(t),
                    mybir.ActivationFunctionType.Square,
                    accum_out=sq[:, t : t + 1],
                )
            nc.vector.tensor_tensor(
                sq3[:], ftile(3), ftile(3), mybir.AluOpType.mult
            )
            nc.vector.tensor_reduce(
                sq[:, 3:4], sq3[:], mybir.AxisListType.X, mybir.AluOpType.add
            )

            nrm = pool.tile([P, T], f32, tag="nrm")
            w = pool.tile([P, T], f32, tag="w")
            wb = pool.tile([P, T], bf16, tag="wb")
            nc.scalar.sqrt(nrm[:], sq[:])
            nc.vector.reciprocal(w[:], nrm[:])
            nc.vector.tensor_copy(wb[:], w[:])

            # ---- g = sum_i w_i f_i as [128, 8]: F-chunks stationary,
            # w moving (1 column), PSUM accumulates over the 4 row-tiles.
            gp = psum.tile([P, 8], f32, tag="gp")
            for c in range(8):
                for t in range(T):
                    nc.tensor.matmul(
                        gp[:, c : c + 1],
                        ftile(t, c * P, (c + 1) * P),
                        wb[:, t : t + 1],
                        start=(t == 0),
                        stop=(t == T - 1),
                    )
            gs = pool.tile([P, 8], bf16, tag="gs")
            nc.vector.tensor_copy(gs[:], gp[:])

            # ---- combine the 8 per-core partials.
            nc.sync.dma_start(g_local.ap(), gs[:])
            if collective:
                nc.gpsimd.collective_compute(
                    "AllReduce",
                    mybir.AluOpType.add,
                    replica_groups=[list(range(N_CORES))],
                    ins=[g_local.ap().opt()],
                    outs=[g_all.ap().opt()],
                )
            # collective=False (timing model): the load below reads g_local
            # instead, keeping the same dependency chain; the AllReduce
            # itself is modeled externally by the test harness (its cost
            # figure is end-to-end, including the collective's own DMAs).
            gsrc = g_all if collective else g_local
            # load the summed g as [8 partitions x 128]
            ga8 = pool.tile([8, P], bf16, tag="ga8")
            nc.sync.dma_start(
                ga8[:], gsrc.ap().rearrange("(a b) n -> a (b n)", a=8)
            )

            # ---- finisher: ||g||^2 -> loss.
            scg = pool.tile([8, P], bf16, tag="scg")
            sqg = pool.tile([8, 1], f32, tag="sqg")
            nc.scalar.activation(
                scg[:],
                ga8[:],
                mybir.ActivationFunctionType.Square,
                accum_out=sqg[:],
            )
            lp = psum.tile([P, 1], f32, tag="lp")
            nc.tensor.matmul(lp[:], ones8[:], sqg[:], start=True, stop=True)
            lossb = pool.tile([P, 1], f32, tag="lossb")
            # loss = ||g||^2 * scale + bias, broadcast on 128 partitions
            nc.scalar.activation(
                lossb[:],
                lp[:],
                mybir.ActivationFunctionType.Copy,
                bias=_LOSS_BIAS,
                scale=_LOSS_SCALE,
            )
            nc.sync.dma_start(loss_out.ap(), lossb[:])

    nc.compile()
    return nc


def _get_nc():
    global _built
    if _built is None:
        _built = _build()
    return _built


def kernel(feats: np.ndarray) -> np.ndarray:
    from concourse import bass_utils

    nc = _get_nc()
    feats = np.ascontiguousarray(np.asarray(feats, dtype=np.float32))
    assert feats.shape == (N, D), feats.shape

    in_maps = [
        {"feats": feats[c * ROWS : (c + 1) * ROWS]} for c in range(N_CORES)
    ]
    res = bass_utils.run_bass_kernel_spmd(
        nc, in_maps, core_ids=list(range(N_CORES))
    )
    out = res.results[0]["loss"]
    return np.float32(out.reshape(-1)[0])


# revision 6
# speedup vs baseline: 1.2390x; 1.0331x over previous
"""Trainium2 Bass kernel for the all-pairs cosine-similarity loss.

Reference computes:  loss = mean_{i<j}(1 - cos(f_i, f_j))
Closed form used here (mathematically identical for nonzero rows):
    u_i = f_i / ||f_i||           (normalized rows)
    g   = sum_i u_i               (D-vector)
    sum_{i<j} cos(f_i,f_j) = (||g||^2 - N) / 2
    loss = 1 - (||g||^2 - N) / (2 * num_pairs)

This turns an O(N^2 D) matmul problem into an O(N D) memory-bound pass.
Per-core (512 rows):
  * 2 input DMAs stream the shard f32->bf16 (SWDGE inline cast).
  * Row norms: ACT square+accum for tiles 0-2, DVE mult+reduce for
    tile 3 (the two engines finish together; one batched sqrt/recip).
  * Weighted row-sum g = sum_i f_i/||f_i|| on PE with the F-chunks as
    STATIONARY [128x128] weights and w as the 1-column moving operand:
    32 one-column matmuls land g directly as [128 partitions x 8] in
    PSUM, so every post-matmul tensor is 128-partition shaped (one-op
    casts, small DMA descriptor counts).
  * g goes to DRAM, AllReduce(add) combines the 8 partial g's, the sum
    comes back as [8 partitions x 128] (a layout permutation, harmless:
    AllReduce is elementwise and the finisher sums all components), and
    a short ACT->PE->ACT chain writes the scalar loss (broadcast over
    128 partitions; python reads [0, 0]).

bf16 note: matmul operands are bf16 but all accumulation is fp32
(PSUM / activation accumulator).  The loss is 1 + O(1e-5); its error
is dominated by the reference's own fp32 rounding, and the bf16
rounding here perturbs it by ~1e-7.
"""

import numpy as np

N = 4096
D = 1024
N_CORES = 8
ROWS = N // N_CORES          # 512 rows per core
P = 128                      # SBUF partitions
T = ROWS // P                # 4 row-tiles of [128, D] per core
NUM_PAIRS = N * (N - 1) // 2

_LOSS_SCALE = -1.0 / (2.0 * NUM_PAIRS)
_LOSS_BIAS = 1.0 + N / (2.0 * NUM_PAIRS)

# Input-tile dtype: fp8(e4m3) halves the modeled input-DMA transfer time
# vs bf16 and is plenty for this loss (error ~5e-7, gate 2e-4).  Flip to
# False to fall back to bf16 tiles.
FP8_INPUT = True

_built = None


def _build(collective: bool = True):
    import concourse.bacc as bacc
    import concourse.mybir as mybir
    import concourse.tile as tile

    f32 = mybir.dt.float32
    bf16 = mybir.dt.bfloat16
    dt_in = mybir.dt.float8e4 if FP8_INPUT else bf16
    nc = bacc.Bacc(
        "TRN2", target_bir_lowering=False, debug=False, num_devices=N_CORES
    )

    feats = nc.dram_tensor("feats", [ROWS, D], f32, kind="ExternalInput")
    # Scalar loss broadcast over 128 partitions; python reads [0, 0].
    loss_out = nc.dram_tensor("loss", [P, 1], f32, kind="ExternalOutput")
    # DRAM bounce buffers for the collective (I/O tensors are not legal
    # collective operands; output must be in the Shared scratchpad).
    g_local = nc.dram_tensor("g_local", [P, 8], bf16)
    g_all = nc.dram_tensor("g_all", [P, 8], bf16, addr_space="Shared")

    with tile.TileContext(nc) as tc:
        with (
            tc.tile_pool(name="pool", bufs=1) as pool,
            tc.tile_pool(name="psum", bufs=1, space="PSUM") as psum,
        ):
            # ---- ACT function-table warm (Square / Sqrt+Copy).  The 95ns
            # memset goes FIRST on Pool so both 1.3us table loads run while
            # the input DMAs stream instead of landing between the squares.
            dummy = pool.tile([1, 1], f32, tag="dummy")
            dummy2 = pool.tile([1, 1], f32, tag="dummy2")
            nc.gpsimd.memset(dummy[:], 1.0)
            nc.scalar.square(dummy[:], dummy[:])
            nc.scalar.sqrt(dummy2[:], dummy[:])

            # ---- input DMAs: rows = h*256 + u*128 + p -> DMA h, tile 2h+u.
            fview = feats.ap().rearrange("(h u p) d -> h p u d", h=2, u=2, p=P)
            fab = []
            for h in range(2):
                ft = pool.tile([P, 2 * D], dt_in, tag=f"f{h}", name=f"f{h}")
                nc.gpsimd.dma_start(
                    ft[:].rearrange("p (u d) -> p u d", u=2), fview[h]
                )
                fab.append(ft)

            def ftile(t, lo=0, hi=D):
                return fab[t // 2][:, (t % 2) * D + lo : (t % 2) * D + hi]

            ones8 = pool.tile([8, P], f32, tag="ones8")
            nc.gpsimd.memset(ones8[:], 1.0)

            # ---- row norms.  ACT: tiles 0-2 (square + free-dim accum);
            # DVE: tile 3 (mult + reduce) so both engines finish together.
            sq = pool.tile([P, T], f32, tag="sq")
            sc = pool.tile([P, D], bf16, tag="sc")
            sq3 = pool.tile([P, D], bf16, tag="sq3")
            for t in range(3):
                nc.scalar.activation(
                    sc[:],
                    ftile(t),
                    mybir.ActivationFunctionType.Square,
                    accum_out=sq[:, t : t + 1],
                )
            nc.vector.tensor_tensor(
                sq3[:], ftile(3), ftile(3), mybir.AluOpType.mult
            )
            nc.vector.tensor_reduce(
                sq[:, 3:4], sq3[:], mybir.AxisListType.X, mybir.AluOpType.add
            )

            nrm = pool.tile([P, T], f32, tag="nrm")
            w = pool.tile([P, T], f32, tag="w")
            wb = pool.tile([P, T], bf16, tag="wb")
            nc.scalar.sqrt(nrm[:], sq[:])
            nc.vector.reciprocal(w[:], nrm[:])
            nc.vector.tensor_copy(wb[:], w[:])

            # ---- g = sum_i w_i f_i as [128, 8]: F-chunks stationary,
            # w moving (1 column), PSUM accumulates over the 4 row-tiles.
            gp = psum.tile([P, 8], f32, tag="gp")
            for c in range(8):
                for t in range(T):
                    nc.tensor.matmul(
                        gp[:, c : c + 1],
                        ftile(t, c * P, (c + 1) * P),
                        wb[:, t : t + 1],
                        start=(t == 0),
                        stop=(t == T - 1),
                    )
            gs = pool.tile([P, 8], bf16, tag="gs")
            nc.vector.tensor_copy(gs[:], gp[:])

            # ---- combine the 8 per-core partials.
            nc.sync.dma_start(g_local.ap(), gs[:])
            if collective:
                nc.gpsimd.collective_compute(
                    "AllReduce",
                    mybir.AluOpType.add,
                    replica_groups=[list(range(N_CORES))],
                    ins=[g_local.ap().opt()],
                    outs=[g_all.ap().opt()],
                )
            # collective=False (timing model): the load below reads g_local
            # instead, keeping the same dependency chain; the AllReduce
            # itself is modeled externally by the test harness (its cost
            # figure is end-to-end, including the collective's own DMAs).
            gsrc = g_all if collective else g_local
            # load the summed g as [8 partitions x 128]
            ga8 = pool.tile([8, P], bf16, tag="ga8")
            nc.sync.dma_start(
                ga8[:], gsrc.ap().rearrange("(a b) n -> a (b n)", a=8)
            )

            # ---- finisher: ||g||^2 -> loss.
            scg = pool.tile([8, P], bf16, tag="scg")
            sqg = pool.tile([8, 1], f32, tag="sqg")
            nc.scalar.activation(
                scg[:],
                ga8[:],
                mybir.ActivationFunctionType.Square,
                accum_out=sqg[:],
            )
            lp = psum.tile([P, 1], f32, tag="lp")
            nc.tensor.matmul(lp[:], ones8[:], sqg[:], start=True, stop=True)
            lossb = pool.tile([P, 1], f32, tag="lossb")
            # loss = ||g||^2 * scale + bias, broadcast on 128 partitions
            nc.scalar.activation(
                lossb[:],
                lp[:],
                mybir.ActivationFunctionType.Copy,
                bias=_LOSS_BIAS,
                scale=_LOSS_SCALE,
            )
            nc.sync.dma_start(loss_out.ap(), lossb[:])

    nc.compile()
    return nc


def _get_nc():
    global _built
    if _built is None:
        _built = _build()
    return _built


def kernel(feats: np.ndarray) -> np.ndarray:
    from concourse import bass_utils

    nc = _get_nc()
    feats = np.ascontiguousarray(np.asarray(feats, dtype=np.float32))
    assert feats.shape == (N, D), feats.shape

    in_maps = [
        {"feats": feats[c * ROWS : (c + 1) * ROWS]} for c in range(N_CORES)
    ]
    res = bass_utils.run_bass_kernel_spmd(
        nc, in_maps, core_ids=list(range(N_CORES))
    )
    out = res.results[0]["loss"]
    return np.float32(out.reshape(-1)[0])


# revision 9
# speedup vs baseline: 1.2562x; 1.0139x over previous
"""Trainium2 Bass kernel for the all-pairs cosine-similarity loss.

Reference computes:  loss = mean_{i<j}(1 - cos(f_i, f_j))
Closed form used here (mathematically identical for nonzero rows):
    u_i = f_i / ||f_i||           (normalized rows)
    g   = sum_i u_i               (D-vector)
    sum_{i<j} cos(f_i,f_j) = (||g||^2 - N) / 2
    loss = 1 - (||g||^2 - N) / (2 * num_pairs)

This turns an O(N^2 D) matmul problem into an O(N D) memory-bound pass.
Per-core (512 rows):
  * 2 input DMAs stream the shard f32->bf16 (SWDGE inline cast).
  * Row norms: ACT square+accum for tiles 0-2, DVE mult+reduce for
    tile 3 (the two engines finish together; one batched sqrt/recip).
  * Weighted row-sum g = sum_i f_i/||f_i|| on PE with the F-chunks as
    STATIONARY [128x128] weights and w as the 1-column moving operand:
    32 one-column matmuls land g directly as [128 partitions x 8] in
    PSUM, so every post-matmul tensor is 128-partition shaped (one-op
    casts, small DMA descriptor counts).
  * g goes to DRAM, AllReduce(add) combines the 8 partial g's, the sum
    comes back as [8 partitions x 128] (a layout permutation, harmless:
    AllReduce is elementwise and the finisher sums all components), and
    a short ACT->PE->ACT chain writes the scalar loss (broadcast over
    128 partitions; python reads [0, 0]).

bf16 note: matmul operands are bf16 but all accumulation is fp32
(PSUM / activation accumulator).  The loss is 1 + O(1e-5); its error
is dominated by the reference's own fp32 rounding, and the bf16
rounding here perturbs it by ~1e-7.
"""

import numpy as np

N = 4096
D = 1024
N_CORES = 8
ROWS = N // N_CORES          # 512 rows per core
P = 128                      # SBUF partitions
T = ROWS // P                # 4 row-tiles of [128, D] per core
NUM_PAIRS = N * (N - 1) // 2

_LOSS_SCALE = -1.0 / (2.0 * NUM_PAIRS)
_LOSS_BIAS = 1.0 + N / (2.0 * NUM_PAIRS)

# Input-tile dtype: fp8(e4m3) halves the modeled input-DMA transfer time
# vs bf16 and is plenty for this loss (error ~5e-7, gate 2e-4).  Flip to
# False to fall back to bf16 tiles.
FP8_INPUT = True

_built = None


def _build(collective: bool = True):
    import concourse.bacc as bacc
    import concourse.mybir as mybir
    import concourse.tile as tile

    f32 = mybir.dt.float32
    bf16 = mybir.dt.bfloat16
    dt_in = mybir.dt.float8e4 if FP8_INPUT else bf16
    nc = bacc.Bacc(
        "TRN2", target_bir_lowering=False, debug=False, num_devices=N_CORES
    )

    feats = nc.dram_tensor("feats", [ROWS, D], f32, kind="ExternalInput")
    # Scalar loss broadcast over 128 partitions; python reads [0, 0].
    loss_out = nc.dram_tensor("loss", [P, 1], f32, kind="ExternalOutput")
    # DRAM bounce buffers for the collective (I/O tensors are not legal
    # collective operands; output must be in the Shared scratchpad).
    g_local = nc.dram_tensor("g_local", [P, 8], bf16)
    g_all = nc.dram_tensor("g_all", [P, 8], bf16, addr_space="Shared")

    with tile.TileContext(nc) as tc:
        with (
            tc.tile_pool(name="pool", bufs=1) as pool,
            tc.tile_pool(name="psum", bufs=1, space="PSUM") as psum,
        ):
            # ---- ACT function-table warm (Square / Sqrt+Copy).  The 95ns
            # memset goes FIRST on Pool so both 1.3us table loads run while
            # the input DMAs stream instead of landing between the squares.
            dummy = pool.tile([1, 1], f32, tag="dummy")
            dummy2 = pool.tile([1, 1], f32, tag="dummy2")
            nc.gpsimd.memset(dummy[:], 1.0)
            nc.scalar.square(dummy[:], dummy[:])
            nc.scalar.sqrt(dummy2[:], dummy[:])

            # ---- input DMAs, split so the ACT square chain starts ASAP:
            # {b0} alone (364ns transfer -> sq0 starts ~400ns earlier), then
            # {b1,b2} (b1 feeds the slower DVE norm path early, b2 is ACT's
            # second square), then {b3} (ACT reaches it last).
            groups = [(0, 1), (1, 2), (3, 1)]
            tview = {}
            for g0, L in groups:
                ft = pool.tile([P, L * D], dt_in, tag=f"f{g0}", name=f"f{g0}")
                src = feats.ap()[g0 * P : (g0 + L) * P].rearrange(
                    "(u p) d -> p u d", p=P
                )
                nc.gpsimd.dma_start(
                    ft[:].rearrange("p (u d) -> p u d", u=L), src
                )
                for i in range(L):
                    tview[g0 + i] = (ft, i)

            def ftile(t, lo=0, hi=D):
                ft, i = tview[t]
                return ft[:, i * D + lo : i * D + hi]

            ones8 = pool.tile([8, P], f32, tag="ones8")
            nc.gpsimd.memset(ones8[:], 1.0)

            # ---- row norms.  ACT: blocks 0,2,3 (square + free-dim accum);
            # DVE: block 1 (mult + reduce) so both engines finish together.
            sq = pool.tile([P, T], f32, tag="sq")
            sc = pool.tile([P, D], bf16, tag="sc")
            sq3 = pool.tile([P, D], bf16, tag="sq3")
            for t in (0, 2, 3):
                nc.scalar.activation(
                    sc[:],
                    ftile(t),
                    mybir.ActivationFunctionType.Square,
                    accum_out=sq[:, t : t + 1],
                )
            nc.vector.tensor_tensor(
                sq3[:], ftile(1), ftile(1), mybir.AluOpType.mult
            )
            nc.vector.tensor_reduce(
                sq[:, 1:2], sq3[:], mybir.AxisListType.X, mybir.AluOpType.add
            )

            # w = sqrt(1/sq): reciprocal (DVE, fp32) then sqrt (ACT) writing
            # the bf16 matmul weights directly — one cross-engine hop and no
            # separate down-cast.
            rsq = pool.tile([P, T], f32, tag="rsq")
            wb = pool.tile([P, T], bf16, tag="wb")
            nc.vector.reciprocal(rsq[:], sq[:])
            nc.scalar.sqrt(wb[:], rsq[:])

            # ---- g = sum_i w_i f_i as [128, 8]: F-chunks stationary,
            # w moving (1 column), PSUM accumulates over the 4 row-tiles.
            gp = psum.tile([P, 8], f32, tag="gp")
            for c in range(8):
                for t in range(T):
                    nc.tensor.matmul(
                        gp[:, c : c + 1],
                        ftile(t, c * P, (c + 1) * P),
                        wb[:, t : t + 1],
                        start=(t == 0),
                        stop=(t == T - 1),
                    )
            gs = pool.tile([P, 8], bf16, tag="gs")
            nc.vector.tensor_copy(gs[:], gp[:])

            # ---- combine the 8 per-core partials.
            nc.sync.dma_start(g_local.ap(), gs[:])
            if collective:
                nc.gpsimd.collective_compute(
                    "AllReduce",
                    mybir.AluOpType.add,
                    replica_groups=[list(range(N_CORES))],
                    ins=[g_local.ap().opt()],
                    outs=[g_all.ap().opt()],
                )
            # collective=False (timing model): the load below reads g_local
            # instead, keeping the same dependency chain; the AllReduce
            # itself is modeled externally by the test harness (its cost
            # figure is end-to-end, including the collective's own DMAs).
            gsrc = g_all if collective else g_local
            # load the summed g as [8 partitions x 128]
            ga8 = pool.tile([8, P], bf16, tag="ga8")
            nc.sync.dma_start(
                ga8[:], gsrc.ap().rearrange("(a b) n -> a (b n)", a=8)
            )

            # ---- finisher: ||g||^2 -> loss.
            scg = pool.tile([8, P], bf16, tag="scg")
            sqg = pool.tile([8, 1], f32, tag="sqg")
            nc.scalar.activation(
                scg[:],
                ga8[:],
                mybir.ActivationFunctionType.Square,
                accum_out=sqg[:],
            )
            lp = psum.tile([P, 1], f32, tag="lp")
            nc.tensor.matmul(lp[:], ones8[:], sqg[:], start=True, stop=True)
            lossb = pool.tile([P, 1], f32, tag="lossb")
            # loss = ||g||^2 * scale + bias, broadcast on 128 partitions
            nc.scalar.activation(
                lossb[:],
                lp[:],
                mybir.ActivationFunctionType.Copy,
                bias=_LOSS_BIAS,
                scale=_LOSS_SCALE,
            )
            nc.sync.dma_start(loss_out.ap(), lossb[:])

    nc.compile()
    return nc


def _get_nc():
    global _built
    if _built is None:
        _built = _build()
    return _built


def kernel(feats: np.ndarray) -> np.ndarray:
    from concourse import bass_utils

    nc = _get_nc()
    feats = np.ascontiguousarray(np.asarray(feats, dtype=np.float32))
    assert feats.shape == (N, D), feats.shape

    in_maps = [
        {"feats": feats[c * ROWS : (c + 1) * ROWS]} for c in range(N_CORES)
    ]
    res = bass_utils.run_bass_kernel_spmd(
        nc, in_maps, core_ids=list(range(N_CORES))
    )
    out = res.results[0]["loss"]
    return np.float32(out.reshape(-1)[0])


# revision 10
# speedup vs baseline: 1.2563x; 1.0001x over previous
"""Trainium2 Bass kernel for the all-pairs cosine-similarity loss.

Reference computes:  loss = mean_{i<j}(1 - cos(f_i, f_j))
Closed form used here (mathematically identical for nonzero rows):
    u_i = f_i / ||f_i||           (normalized rows)
    g   = sum_i u_i               (D-vector)
    sum_{i<j} cos(f_i,f_j) = (||g||^2 - N) / 2
    loss = 1 - (||g||^2 - N) / (2 * num_pairs)

This turns an O(N^2 D) matmul problem into an O(N D) memory-bound pass.
Per-core (512 rows):
  * 2 input DMAs stream the shard f32->bf16 (SWDGE inline cast).
  * Row norms: ACT square+accum for tiles 0-2, DVE mult+reduce for
    tile 3 (the two engines finish together; one batched sqrt/recip).
  * Weighted row-sum g = sum_i f_i/||f_i|| on PE with the F-chunks as
    STATIONARY [128x128] weights and w as the 1-column moving operand:
    32 one-column matmuls land g directly as [128 partitions x 8] in
    PSUM, so every post-matmul tensor is 128-partition shaped (one-op
    casts, small DMA descriptor counts).
  * g goes to DRAM, AllReduce(add) combines the 8 partial g's, the sum
    comes back as [8 partitions x 128] (a layout permutation, harmless:
    AllReduce is elementwise and the finisher sums all components), and
    a short ACT->PE->ACT chain writes the scalar loss (broadcast over
    128 partitions; python reads [0, 0]).

bf16 note: matmul operands are bf16 but all accumulation is fp32
(PSUM / activation accumulator).  The loss is 1 + O(1e-5); its error
is dominated by the reference's own fp32 rounding, and the bf16
rounding here perturbs it by ~1e-7.
"""

import numpy as np

N = 4096
D = 1024
N_CORES = 8
ROWS = N // N_CORES          # 512 rows per core
P = 128                      # SBUF partitions
T = ROWS // P                # 4 row-tiles of [128, D] per core
NUM_PAIRS = N * (N - 1) // 2

_LOSS_SCALE = -1.0 / (2.0 * NUM_PAIRS)
_LOSS_BIAS = 1.0 + N / (2.0 * NUM_PAIRS)

# Input-tile dtype: fp8(e4m3) halves the modeled input-DMA transfer time
# vs bf16 and is plenty for this loss (error ~5e-7, gate 2e-4).  Flip to
# False to fall back to bf16 tiles.
FP8_INPUT = True

_built = None


def _build(collective: bool = True):
    import concourse.bacc as bacc
    import concourse.mybir as mybir
    import concourse.tile as tile

    f32 = mybir.dt.float32
    bf16 = mybir.dt.bfloat16
    dt_in = mybir.dt.float8e4 if FP8_INPUT else bf16
    nc = bacc.Bacc(
        "TRN2", target_bir_lowering=False, debug=False, num_devices=N_CORES
    )

    feats = nc.dram_tensor("feats", [ROWS, D], f32, kind="ExternalInput")
    # Scalar loss broadcast over 128 partitions; python reads [0, 0].
    loss_out = nc.dram_tensor("loss", [P, 1], f32, kind="ExternalOutput")
    # DRAM bounce buffers for the collective (I/O tensors are not legal
    # collective operands; output must be in the Shared scratchpad).
    g_local = nc.dram_tensor("g_local", [P, 8], bf16)
    g_all = nc.dram_tensor("g_all", [P, 8], bf16, addr_space="Shared")

    with tile.TileContext(nc) as tc:
        with (
            tc.tile_pool(name="pool", bufs=1) as pool,
            tc.tile_pool(name="psum", bufs=1, space="PSUM") as psum,
        ):
            # ---- ACT function-table warm (Square / Sqrt+Copy).  The 95ns
            # memset goes FIRST on Pool so both 1.3us table loads run while
            # the input DMAs stream instead of landing between the squares.
            dummy = pool.tile([1, 1], f32, tag="dummy")
            dummy2 = pool.tile([1, 1], f32, tag="dummy2")
            nc.gpsimd.memset(dummy[:], 1.0)
            nc.scalar.square(dummy[:], dummy[:])
            nc.scalar.sqrt(dummy2[:], dummy[:])

            # ---- input DMAs, split so the ACT square chain starts ASAP:
            # {b0} alone (364ns transfer -> sq0 starts ~400ns earlier), then
            # {b1,b2} (b1 feeds the slower DVE norm path early, b2 is ACT's
            # second square), then {b3} (ACT reaches it last).
            groups = [(0, 1), (1, 2), (3, 1)]
            tview = {}
            for g0, L in groups:
                ft = pool.tile([P, L * D], dt_in, tag=f"f{g0}", name=f"f{g0}")
                src = feats.ap()[g0 * P : (g0 + L) * P].rearrange(
                    "(u p) d -> p u d", p=P
                )
                nc.gpsimd.dma_start(
                    ft[:].rearrange("p (u d) -> p u d", u=L), src
                )
                for i in range(L):
                    tview[g0 + i] = (ft, i)

            def ftile(t, lo=0, hi=D):
                ft, i = tview[t]
                return ft[:, i * D + lo : i * D + hi]

            ones8 = pool.tile([8, P], f32, tag="ones8")
            nc.gpsimd.memset(ones8[:], 1.0)

            # ---- row norms.  ACT: blocks 0,2,3 (square + free-dim accum);
            # DVE: block 1 (mult + reduce) so both engines finish together.
            sq = pool.tile([P, T], f32, tag="sq")
            sc = pool.tile([P, D], bf16, tag="sc")
            sq3 = pool.tile([P, D], bf16, tag="sq3")
            for t in (0, 2, 3):
                nc.scalar.activation(
                    sc[:],
                    ftile(t),
                    mybir.ActivationFunctionType.Square,
                    accum_out=sq[:, t : t + 1],
                )
            nc.vector.tensor_tensor(
                sq3[:], ftile(1), ftile(1), mybir.AluOpType.mult
            )
            nc.vector.tensor_reduce(
                sq[:, 1:2], sq3[:], mybir.AxisListType.X, mybir.AluOpType.add
            )

            # w = sqrt(1/sq): reciprocal (DVE, fp32) then sqrt (ACT) writing
            # the bf16 matmul weights directly — one cross-engine hop and no
            # separate down-cast.
            rsq = pool.tile([P, T], f32, tag="rsq")
            wb = pool.tile([P, T], bf16, tag="wb")
            nc.vector.reciprocal(rsq[:], sq[:])
            nc.scalar.sqrt(wb[:], rsq[:])

            # ---- g = sum_i w_i f_i as [128, 8]: F-chunks stationary,
            # w moving (1 column), PSUM accumulates over the 4 row-tiles.
            gp = psum.tile([P, 8], f32, tag="gp")
            for c in range(8):
                for t in range(T):
                    nc.tensor.matmul(
                        gp[:, c : c + 1],
                        ftile(t, c * P, (c + 1) * P),
                        wb[:, t : t + 1],
                        start=(t == 0),
                        stop=(t == T - 1),
                    )
            gs = pool.tile([P, 8], bf16, tag="gs")
            nc.vector.tensor_copy(gs[:], gp[:])

            # ---- combine the 8 per-core partials.
            nc.sync.dma_start(g_local.ap(), gs[:])
            if collective:
                nc.gpsimd.collective_compute(
                    "AllReduce",
                    mybir.AluOpType.add,
                    replica_groups=[list(range(N_CORES))],
                    ins=[g_local.ap().opt()],
                    outs=[g_all.ap().opt()],
                )
            # collective=False (timing model): the load below reads g_local
            # instead, keeping the same dependency chain; the AllReduce
            # itself is modeled externally by the test harness (its cost
            # figure is end-to-end, including the collective's own DMAs).
            gsrc = g_all if collective else g_local
            # load the summed g as [8 partitions x 128]
            ga8 = pool.tile([8, P], bf16, tag="ga8")
            nc.sync.dma_start(
                ga8[:], gsrc.ap().rearrange("(a b) n -> a (b n)", a=8)
            )

            # ---- finisher: ||g||^2 -> loss.  DVE mult+reduce (2x mode on
            # the bf16 square) beats ACT square+accum-read here.
            scg = pool.tile([8, P], bf16, tag="scg")
            sqg = pool.tile([8, 1], f32, tag="sqg")
            nc.vector.tensor_tensor(
                scg[:], ga8[:], ga8[:], mybir.AluOpType.mult
            )
            nc.vector.tensor_reduce(
                sqg[:], scg[:], mybir.AxisListType.X, mybir.AluOpType.add
            )
            lp = psum.tile([P, 1], f32, tag="lp")
            nc.tensor.matmul(lp[:], ones8[:], sqg[:], start=True, stop=True)
            lossb = pool.tile([P, 1], f32, tag="lossb")
            # loss = ||g||^2 * scale + bias, broadcast on 128 partitions
            nc.scalar.activation(
                lossb[:],
                lp[:],
                mybir.ActivationFunctionType.Copy,
                bias=_LOSS_BIAS,
                scale=_LOSS_SCALE,
            )
            nc.sync.dma_start(loss_out.ap(), lossb[:])

    nc.compile()
    return nc


def _get_nc():
    global _built
    if _built is None:
        _built = _build()
    return _built


def kernel(feats: np.ndarray) -> np.ndarray:
    from concourse import bass_utils

    nc = _get_nc()
    feats = np.ascontiguousarray(np.asarray(feats, dtype=np.float32))
    assert feats.shape == (N, D), feats.shape

    in_maps = [
        {"feats": feats[c * ROWS : (c + 1) * ROWS]} for c in range(N_CORES)
    ]
    res = bass_utils.run_bass_kernel_spmd(
        nc, in_maps, core_ids=list(range(N_CORES))
    )
    out = res.results[0]["loss"]
    return np.float32(out.reshape(-1)[0])
